# revision 13
# baseline (speedup 1.0000x reference)
"""Multi-head attention (B=2, N=2048, D=1024, H=16, hd=64) on 8 trn2 NeuronCores.

Sharding: 8 cores = 2 (batch) x 4 (head groups of 4 heads).
Core c: batch b = c // 4, heads hg*4 .. hg*4+3 where hg = c % 4.

Per-core program (identical SPMD program, per-core data):
  inputs (DRAM):
    xT     [1024, 2048]  = x[b].T
    wqkT   [1024, 512]   = w_qkv[[q rows, k rows] of local heads].T
    wvT    [1024, 256]   = w_qkv[v rows of local heads].T
    wprojT [256, 1024]   = w_proj[:, local head cols].T
  output:
    out    [2048, 1024]  partial (row-parallel) projection output

  1) qkT  [512, 2048]  = wqkT.T @ xT        (q,k in transposed layout)
     v    [2048, 256]  = (xT.T @ wvT)       (natural layout, ones-augmented)
  2) per (q-block 512, head): scores_T [keys, q] = kT.T @ qT, exp on ACT
     (scale 1/8 fused), PV with ones-augmented V gives unnormalized out_T
     [64, q] + denominator row; reciprocal + K=1 outer-product matmul
     broadcast + DVE multiply normalizes.
  3) partial = attn_out_T.T @ wprojT  -> [2048, 1024]

Host unshard: out[b] = sum over 4 head-group partials + b_proj.
"""

import sys

if "/opt/trn_rl_repo" not in sys.path:
    sys.path.insert(0, "/opt/trn_rl_repo")

import numpy as np

B, N, D, H, HD = 2, 2048, 1024, 16, 64
NCORES = 8
HPC = 4               # heads per core
LQK = HPC * HD        # 256 local q (or k) rows
SCALE = HD ** -0.5    # 0.125

_CACHE = {}


def _build_program():
    import concourse.tile as tile
    from concourse import bacc, mybir

    F32 = mybir.dt.float32
    F32R = mybir.dt.float32r
    BF16 = mybir.dt.bfloat16
    Exp = mybir.ActivationFunctionType.Exp

    nc = bacc.Bacc("TRN2", target_bir_lowering=False, debug=False,
                   num_devices=NCORES)

    xT_d = nc.dram_tensor("xT", [D, N], BF16, kind="ExternalInput").ap()
    wqkT_d = nc.dram_tensor("wqkT", [D, 2 * LQK], BF16, kind="ExternalInput").ap()
    wvT_d = nc.dram_tensor("wvT", [D, LQK], BF16, kind="ExternalInput").ap()
    wprojT_d = nc.dram_tensor("wprojT", [LQK, D], BF16, kind="ExternalInput").ap()
    out_d = nc.dram_tensor("out", [N, D], F32, kind="ExternalOutput").ap()

    KT = D // 128        # 8 contraction tiles for qkv gemms
    NB = N // 512        # 4 seq blocks
    NT = N // 128        # 16 seq tiles
    r = lambda ap: ap  # tiles already f32r

    with tile.TileContext(nc) as tc:
        with (
            nc.allow_low_precision(reason="fp32r matmul operands"),
            tc.tile_pool(name="const", bufs=1) as cpool,
            tc.tile_pool(name="w", bufs=1) as wpool,
            tc.tile_pool(name="x", bufs=1) as xpool,
            tc.tile_pool(name="qk", bufs=1) as qkpool,
            tc.tile_pool(name="vaug", bufs=1) as vapool,
            tc.tile_pool(name="ao", bufs=1) as aopool,
            tc.tile_pool(name="probs", bufs=3) as prpool,
            tc.tile_pool(name="small", bufs=5) as smpool,
            tc.tile_pool(name="stage", bufs=3) as stpool,
            tc.tile_pool(name="psbig", bufs=2, space="PSUM") as psbig,
            tc.tile_pool(name="pspv", bufs=2, space="PSUM") as pspv,
        ):
            ones_f32 = cpool.tile([128, 128], F32)
            nc.vector.memset(ones_f32[:, :], 1.0)
            ones_sb = cpool.tile([65, 128], F32R)
            nc.vector.tensor_copy(ones_sb[:, :], ones_f32[0:65, :])

            # ---- input DMAs (kt-chunked so compute starts early) ----
            x_sb = xpool.tile([128, KT, N], BF16)
            wqk_sb = wpool.tile([128, KT, 2 * LQK], BF16)
            wv_sb = wpool.tile([128, KT, LQK], BF16)
            xT_r = xT_d.rearrange("(kt p) n -> p kt n", p=128)
            wqkT_r = wqkT_d.rearrange("(kt p) m -> p kt m", p=128)
            wvT_r = wvT_d.rearrange("(kt p) m -> p kt m", p=128)
            for kt in range(KT):
                nc.sync.dma_start(out=wqk_sb[:, kt, :], in_=wqkT_r[:, kt, :])
                nc.sync.dma_start(out=x_sb[:, kt, :], in_=xT_r[:, kt, :])
                nc.sync.dma_start(out=wv_sb[:, kt, :], in_=wvT_r[:, kt, :])

            # ---- qkT = wqkT.T @ xT : [512, 2048], m-tiles of 128 ----
            # qk_sb m-tile layout: m=0: q heads 0,1 / m=1: q heads 2,3
            #                      m=2: k heads 0,1 / m=3: k heads 2,3
            qk_sb = qkpool.tile([128, 4, N], BF16)

            def qk_gemm(m):
                # kt-outer so 4 seq-blocks share each loaded weight tile
                wm = 0 if m < 2 else 2 * LQK // 2   # q cols 0..255, k cols 256..511
                wcol = wm + (m % 2) * 128
                pss = [psbig.tile([128, 1024], F32, tag="big", name=f"qkps{m}_{i}")
                       for i in range(2)]
                for kt in range(KT):
                    for nb in range(NB):
                        nc.tensor.matmul(
                            pss[nb // 2][:, (nb % 2) * 512:(nb % 2 + 1) * 512],
                            r(wqk_sb[:, kt, wcol:wcol + 128]),
                            r(x_sb[:, kt, nb * 512:(nb + 1) * 512]),
                            start=(kt == 0), stop=(kt == KT - 1),
                        )
                for half in range(2):
                    nc.vector.tensor_copy(
                        qk_sb[:, m, half * 1024:(half + 1) * 1024], pss[half][:, :])

            # ---- v natural [2048, 256] ones-augmented: [128, st, h, 65] ----
            v_sb = vapool.tile([128, NT, HPC, HD + 1], BF16)

            def v_gemm(st):
                ps = pspv.tile([128, 1024], F32, tag="pv")
                for kt in range(KT):
                    nc.tensor.matmul(
                        ps[:, 0:LQK],
                        r(x_sb[:, kt, st * 128:(st + 1) * 128]),
                        r(wv_sb[:, kt, :]),
                        start=(kt == 0), stop=(kt == KT - 1),
                    )
                nc.vector.tensor_copy(
                    v_sb[:, st, :, 0:HD],
                    ps[:, 0:LQK].rearrange("p (h d) -> p h d", h=HPC))
                nc.vector.tensor_copy(
                    v_sb[:, st, :, HD:HD + 1],
                    ones_f32[:, 0:HPC].rearrange("p (h c) -> p h c", c=1))

            # deps of attention chain (h0, qh0) first so ACT starts early;
            # the v gemm rides inside the first chain, qk m=3/1 fill the
            # PE gaps of the first two ACT-paced chains
            for m in (2, 0):
                qk_gemm(m)

            # late small DMA for proj weights
            wproj_sb = wpool.tile([128, 2, D], BF16)
            nc.sync.dma_start(
                out=wproj_sb[:, :, :],
                in_=wprojT_d.rearrange("(kt p) o -> p kt o", p=128))

            # ---- attention + projection, q-block major ----
            ao_sb = aopool.tile([128, 2, N], BF16)   # proj lhsT kt0: heads 0,1; kt1: heads 2,3

            def attn_chain(h, qh, with_v=False):
                """Scores -> exp -> PV over a 1024-wide q block; recip emitted
                inline so the DVE computes it while the PE runs the next chain.
                with_v interleaves the v gemm (one seq-tile per key-tile) into
                the first chain so attention starts without waiting for v."""
                pi = (h % 2) * 64
                mq, mk = h // 2, 2 + h // 2
                qT = qk_sb[pi:pi + 64, mq, qh * 1024:(qh + 1) * 1024]
                pv = pspv.tile([128, 1024], F32, tag="pv")
                for kt in range(2 * KT):
                    sc = psbig.tile([128, 1024], F32, tag="big")
                    pr = prpool.tile([128, 1024], BF16, tag="probs")
                    kT = qk_sb[pi:pi + 64, mk, kt * 128:(kt + 1) * 128]
                    for j in range(2):   # N=512 halves sharing the kT weights
                        nc.tensor.matmul(
                            sc[:, j * 512:(j + 1) * 512], r(kT),
                            qT[:, j * 512:(j + 1) * 512],
                            start=True, stop=True)
                    nc.scalar.activation(pr[:, :], sc[:, :], Exp, scale=SCALE)
                    if with_v:
                        v_gemm(kt)
                    for j in range(2):   # halves sharing the v weights
                        nc.tensor.matmul(
                            pv[0:65, j * 512:(j + 1) * 512],
                            r(v_sb[:, kt, h, 0:HD + 1]),
                            r(pr[:, j * 512:(j + 1) * 512]),
                            start=(kt == 0), stop=(kt == 2 * KT - 1),
                        )
                recip = smpool.tile([65, 1024], F32R, tag="recip")
                nc.vector.reciprocal(recip[64:65, :], pv[64:65, :])
                return pv, recip

            def norm_tail(h, qh, pv, recip):
                """PE outer-product broadcast of recip, DVE multiply, DMA to
                the proj-layout partitions of ao_sb."""
                pi = (h % 2) * 64
                bc = psbig.tile([64, 1024], F32, tag="big")
                for j in range(2):
                    nc.tensor.matmul(bc[:, j * 512:(j + 1) * 512],
                                     r(ones_sb[64:65, 0:64]),
                                     recip[64:65, j * 512:(j + 1) * 512],
                                     start=True, stop=True)
                bcs = smpool.tile([64, 1024], F32, tag="bcs")
                nc.vector.tensor_copy(bcs[:, :], bc[:, :])
                aos = stpool.tile([64, 1024], BF16, tag="aos")
                nc.vector.tensor_mul(aos[:, :], pv[0:64, :], bcs[:, :])
                nc.sync.dma_start(
                    out=ao_sb[pi:pi + 64, h // 2, qh * 1024:(qh + 1) * 1024],
                    in_=aos[:, :])

            def proj(nt):
                outst = stpool.tile([128, D], F32, tag="outst")
                ps = psbig.tile([128, 1024], F32, tag="big")
                for kt2 in range(2):
                    for ob in range(2):   # halves sharing the ao weights
                        nc.tensor.matmul(
                            ps[:, ob * 512:(ob + 1) * 512],
                            r(ao_sb[:, kt2, nt * 128:(nt + 1) * 128]),
                            r(wproj_sb[:, kt2, ob * 512:(ob + 1) * 512]),
                            start=(kt2 == 0), stop=(kt2 == 1),
                        )
                nc.vector.tensor_copy(outst[:, :], ps[:, :])
                nc.sync.dma_start(
                    out=out_d[nt * 128:(nt + 1) * 128, :], in_=outst[:, :])

            # chain schedule: tail(i) lands right after chain(i+1) so the PE
            # never waits on the DVE reciprocal; proj of a q-half interleaves
            # with the next half's chains
            chains = [(h, qh) for qh in range(2) for h in range(HPC)]
            pend = {}
            for i, (h, qh) in enumerate(chains):
                pend[(h, qh)] = attn_chain(h, qh, with_v=(i == 0))
                if i == 1:
                    for m in (3, 1):
                        qk_gemm(m)
                if i >= 1:
                    ph, pqh = chains[i - 1]
                    norm_tail(ph, pqh, *pend.pop((ph, pqh)))
                if i == 5:
                    for nt in range(0, 8):       # proj of q-half 0
                        proj(nt)
            norm_tail(*chains[-1], *pend.pop(chains[-1]))
            for nt in range(8, 16):              # proj of q-half 1
                proj(nt)

    nc.compile()
    return nc


def _get_program():
    if "nc" not in _CACHE:
        _CACHE["nc"] = _build_program()
    return _CACHE["nc"]


def _make_in_maps(x, w_qkv, w_proj):
    import ml_dtypes
    bf16 = ml_dtypes.bfloat16
    x = np.asarray(x, dtype=np.float32)
    w_qkv = np.asarray(w_qkv, dtype=np.float32)
    w_proj = np.asarray(w_proj, dtype=np.float32)
    xT = [np.ascontiguousarray(x[b].T).astype(bf16) for b in range(B)]
    in_maps = []
    for c in range(NCORES):
        b, hg = c // 4, c % 4
        rows = slice(hg * LQK, (hg + 1) * LQK)
        qk_rows = np.r_[np.arange(hg * LQK, (hg + 1) * LQK),
                        D + np.arange(hg * LQK, (hg + 1) * LQK)]
        in_maps.append({
            "xT": xT[b],
            "wqkT": np.ascontiguousarray(w_qkv[qk_rows, :].T).astype(bf16),
            "wvT": np.ascontiguousarray(
                w_qkv[2 * D + np.arange(hg * LQK, (hg + 1) * LQK), :].T).astype(bf16),
            "wprojT": np.ascontiguousarray(w_proj[:, rows].T).astype(bf16),
        })
    return in_maps


def kernel(x, w_qkv, w_proj, b_proj, _return_results=False, _trace=False):
    from concourse import bass_utils

    nc = _get_program()
    in_maps = _make_in_maps(x, w_qkv, w_proj)
    res = bass_utils.run_bass_kernel_spmd(
        nc, in_maps, list(range(NCORES)), trace=_trace)
    partials = np.stack([res.results[c]["out"] for c in range(NCORES)])
    out = partials.reshape(B, 4, N, D).sum(axis=1, dtype=np.float32)
    out = out + np.asarray(b_proj, dtype=np.float32)[None, None, :]
    out = out.astype(np.float32)
    if _return_results:
        return out, res
    return out
